# revision 5
# baseline (speedup 1.0000x reference)
"""Elementwise hard-clip kernel for Trainium2 (8 NeuronCores, SPMD).

Computes y = clip(x, -0.5, 0.5) for x of shape (32, 2, 1048576) float32.

Strategy: flatten to 67,108,864 elements, shard contiguously across 8
cores (8,388,608 elements = 32 MiB per core).  Each core streams tiles of
[128 partitions x FREE] f32 through SBUF: HWDGE load on the SP ring, one
fused VectorE tensor_scalar (min hi, then max lo) per tile, HWDGE store
on the ACT ring.  Memory-bound: ~64 MiB through the SBUF AXI fabric per
core (~435 GB/s ceiling -> ~155 us floor).

Raw bass (no TileContext): hand-rolled semaphore pipeline avoids Tile's
~8 us EVSEM exit barrier and part of its preamble.
"""

import numpy as np

import concourse.bass as bass
import concourse.mybir as mybir
from concourse.bass_utils import run_bass_kernel_spmd

N_CORES = 8
FULL_SHAPE = (32, 2, 1048576)
TOTAL = FULL_SHAPE[0] * FULL_SHAPE[1] * FULL_SHAPE[2]  # 67,108,864
PER_CORE = TOTAL // N_CORES  # 8,388,608
P = 128
FREE = 2048  # elements per partition per tile -> 1 MiB tiles
NTILES = PER_CORE // (P * FREE)  # 32
BUFS = 16

LO = -0.5
HI = 0.5

_nc_cache = None


def _build():
    nc = bass.Bass(target_bir_lowering=False)
    x = nc.dram_tensor("x", [PER_CORE], mybir.dt.float32, kind="ExternalInput")
    y = nc.dram_tensor("y", [PER_CORE], mybir.dt.float32, kind="ExternalOutput")
    xt = x[:].rearrange("(n p f) -> n p f", p=P, f=FREE)
    yt = y[:].rearrange("(n p f) -> n p f", p=P, f=FREE)

    with (
        nc.Block(no_gpsimd_drain=True) as block,
        nc.semaphore("ld") as ld,
        nc.semaphore("cp") as cp,
        nc.semaphore("st") as st,
        nc.sbuf_tensor("buf", [P, FREE * BUFS], mybir.dt.float32) as buf,
    ):
        def slot(i):
            j = i % BUFS
            return buf[:, j * FREE : (j + 1) * FREE]

        @block.sync
        def _(sync):
            for i in range(NTILES):
                if i >= BUFS:
                    # WAR: slot reused; wait for its store to land
                    sync.wait_ge(st, 16 * (i - BUFS + 1))
                sync.dma_start(slot(i), xt[i]).then_inc(ld, 16)

        @block.vector
        def _(vector):
            for i in range(NTILES):
                vector.wait_ge(ld, 16 * (i + 1))
                s = slot(i)
                vector.tensor_scalar(
                    s, s, HI, LO, mybir.AluOpType.min, mybir.AluOpType.max
                ).then_inc(cp, 1)

        @block.scalar
        def _(scalar):
            for i in range(NTILES):
                scalar.wait_ge(cp, i + 1)
                scalar.dma_start(yt[i], slot(i)).then_inc(st, 16)
            # ensure the NEFF doesn't retire with stores still in flight
            scalar.wait_ge(st, 16 * NTILES)

    nc.finalize()
    return nc


def kernel(x):
    global _nc_cache
    x = np.asarray(x, dtype=np.float32)
    shards = np.ascontiguousarray(x).reshape(N_CORES, PER_CORE)
    if _nc_cache is None:
        _nc_cache = _build()
    res = run_bass_kernel_spmd(
        _nc_cache,
        [{"x": shards[i]} for i in range(N_CORES)],
        core_ids=list(range(N_CORES)),
    )
    out = np.concatenate([r["y"] for r in res.results])
    return out.reshape(FULL_SHAPE)
